# revision 93
# baseline (speedup 1.0000x reference)
"""Trainium2 Bass kernel for AttentionProlongationGNN (v2).

Contract: kernel(**inputs) takes FULL unsharded numpy inputs (keys as in
setup_inputs) and returns the FULL (N, 1) float32 output.

Strategy (8 NeuronCores, SPMD single program):
- Nodes sharded 6250/core (padded to 6272 = 49*128 rows).  Edges sharded by
  dst core, grouped into 49 dst-blocks of 128 nodes.
- Per layer each core computes K|V (fp8e4 [nsh, 512B] rows) and Q (bf16,
  kept resident in SBUF).  K|V is AllGathered in THREE node-range chunks so
  each chunk's collective overlaps the still-running block pipeline, every
  gathered table stays < 32768 rows (int16 dma_gather indices), and each
  (block, chunk) edge segment fits one <=1024-index dma_gather call (the
  per-call cost is ~6us flat: Q7 descriptor emission at ~7ns/row plus a
  fixed issue/completion tail, so fewer+bigger calls win).
- Q[dst] is NOT gathered: dst rows are block-local, so qg = mT @ Q_block is
  a per-tile PE matmul against a host-precomputed one-hot mT (in the blob),
  with ACT copying PSUM->SBUF.  This halves the Q7 descriptor-emission load
  vs gathering Q rows per edge (the v1 bottleneck).
- Edge phase per dst-block: one dma_gather per chunk pulls K|V fp8 rows for
  the block's edges.  Dot products / softmax / weighting are block-batched
  DVE ops arranged so no operand is a stride-0 broadcast (broadcasts drop
  DVE from 2 to 1 elem/cycle; the head-broadcast exp is produced on ScalarE
  instead, into a bf16 tile - exp values stay bf16, only K|V storage is
  fp8).  Weighted V and exp(attn) share one [P, t, 264] tile so the
  segment-sum is a SINGLE PE matmul per 128-edge tile (one weight load of
  the is_equal selection matrix), accumulated in PSUM.
- Post phase fused into the same block loop: Wm/residual/LayerNorm and
  next layer's QKV run right after each block drains; h^T and the Q table
  stay resident in SBUF.  Wo and bo are folded into the second half of Wm
  on the host (linear before the ReLU), removing the Wo matmul and one
  transpose pair from the critical chain.  LayerNorm gets mean/var from
  bn_stats+bn_aggr (2 DVE ops), rstd via Sqrt+reciprocal, and a fused
  (x-mu)*rstd on ScalarE.  ReLUs run on DVE (tensor_scalar_max) to cut
  ACT table reloads.
- All matmuls bf16 (fp32 matmul runs 2-pass LOW_HIGH at ~4x the cost).
"""
import sys

if "/opt/trn_rl_repo" not in sys.path:
    sys.path.insert(0, "/opt/trn_rl_repo")

import numpy as np
import ml_dtypes

from concourse import bass, mybir, bacc, tile
from concourse.masks import make_identity
from concourse.bass_utils import run_bass_kernel_spmd

FP = mybir.dt.float32
BF = mybir.dt.bfloat16
F8 = mybir.dt.float8e4
I16 = mybir.dt.int16
AF = mybir.ActivationFunctionType
OP = mybir.AluOpType

P = 128
NCORES = 8
H = 256
NH = 8
HD = H // NH
ED = 3
F_IN = 10
L = 3
EPS_LN = 1e-5
HC = H // P            # feature chunks (2)
BF_NP = ml_dtypes.bfloat16
F8_NP = ml_dtypes.float8_e4m3


# ---------------------------------------------------------------- host prep

def chunk_bounds(blk):
    """Node-block boundaries for the chunked K|V AllGather.

    Chunks are greedily sized to the int16 dma_gather index limit
    (32 blocks = 4096 rows x 8 cores = 32768 table rows, max index
    32767): fewer chunks mean fewer padded edge segments per block
    (less work on every edge-phase engine) and fewer AllGathers."""
    bounds = [0]
    while bounds[-1] < blk:
        bounds.append(min(bounds[-1] + 32, blk))
    return bounds


def ag_segments(bounds, blk):
    """AllGather segments per chunk (one per chunk: a Shared-DRAM
    collective output must have a single writer instruction)."""
    nch = len(bounds) - 1
    return [[(bounds[i], bounds[i + 1])] for i in range(nch)]


def edge_schedule(edge_index, N):
    """Per-core edge schedule with uniform (cross-core identical) tiling.

    Edges are owned by the core holding their dst.  Within each dst-block of
    128 nodes the edges are segmented by which AllGather chunk their src row
    falls in; each segment is padded to whole 128-edge tiles, tile counts
    maxed across cores so the single SPMD program fits all cores.
    """
    nsh = N // NCORES
    blk = (nsh + P - 1) // P
    bounds = chunk_bounds(blk)
    nch = len(bounds) - 1
    lo = [bounds[i] * P for i in range(nch)]
    hi = [bounds[i + 1] * P for i in range(nch)]
    rows_c = [hi[i] - lo[i] for i in range(nch)]
    for r in rows_c:
        assert r * NCORES <= 32768  # max int16 index 32767

    src = edge_index[0].astype(np.int64)
    dst = edge_index[1].astype(np.int64)
    core_of = dst // nsh
    scr = src // nsh                  # owning core of src
    srl = src % nsh                   # local row of src
    chunk_of = np.zeros_like(srl)
    for i in range(nch):
        chunk_of[(srl >= lo[i]) & (srl < hi[i])] = i
    # table row layout follows the AG segments: [core0 segA | core1 segA |
    # ... | core0 segB | ...] per chunk
    segs = ag_segments(bounds, blk)
    rowin = np.zeros_like(srl)
    for i in range(nch):
        base = 0
        for (s0, s1) in segs[i]:
            lo_r, hi_r = s0 * P, s1 * P
            seg_rows = hi_r - lo_r
            sel = (srl >= lo_r) & (srl < hi_r)
            rowin[sel] = base + scr[sel] * seg_rows + (srl[sel] - lo_r)
            base += NCORES * seg_rows

    per_core = []
    nseg = np.zeros((NCORES, blk, nch), np.int64)
    for c in range(NCORES):
        eids = np.where(core_of == c)[0]
        ld = dst[eids] - c * nsh
        b = ld // P
        order = np.lexsort((chunk_of[eids], b))
        eids = eids[order]
        b = b[order]
        per_core.append((eids, b))
        for blki in range(blk):
            be = eids[b == blki]
            for i in range(nch):
                nseg[c, blki, i] = (chunk_of[be] == i).sum()
    nmax = nseg.max(axis=0).astype(np.int64)                        # [blk, nch]
    Tc = np.maximum(0, -(-nmax // P)).astype(np.int64)              # [blk, nch]
    zero = Tc.sum(axis=1) == 0
    Tc[zero, 0] = 1
    nmax = np.where(Tc > 0, np.maximum(1, nmax), 0)
    T = Tc.sum(axis=1)
    T_tot = int(T.sum())
    toff = np.concatenate([[0], np.cumsum(T)])[:-1]

    cores = []
    for c in range(NCORES):
        eids, b = per_core[c]
        idxc = [np.zeros((blk, max(1, int(Tc[:, i].max())) * P), np.int16)
                for i in range(nch)]
        dl = np.full((P, T_tot), -1.0, np.float32)
        esel = np.full(T_tot * P, -1, np.int64)
        for blki in range(blk):
            be = eids[b == blki]
            base_slot = 0
            pos = 0
            for i in range(nch):
                n = int(nseg[c, blki, i])
                seg = be[pos:pos + n]
                pos += n
                if n:
                    idxc[i][blki, :n] = rowin[seg].astype(np.int16)
                    j = base_slot + np.arange(n)
                    pp, tt = j % P, j // P
                    ldl = dst[seg] - c * nsh - blki * P
                    dl[pp, toff[blki] + tt] = ldl.astype(np.float32)
                    esel[(toff[blki] + tt) * P + pp] = seg
                base_slot += int(Tc[blki, i]) * P
        cores.append(dict(idxc=idxc, dl=dl, esel=esel))
    meta = dict(nsh=nsh, blk=blk, bounds=bounds, nch=nch, rows_c=rows_c,
                segs=segs, Tc=Tc, T=T, toff=toff, T_tot=T_tot, nmax=nmax)
    return meta, cores


def wrap16(vals_int16, ntiles):
    """[ntiles*128] slot-ordered indices -> dma_gather layout [128, ntiles*8]:
    wrapped into 16 partitions and replicated across the 8 Q7-core stripes."""
    if ntiles == 0:
        return np.zeros((P, 0), np.int16)
    out = np.zeros((16, ntiles * 8), np.int16)
    j = np.arange(ntiles * P)
    out[j % 16, j // 16] = vals_int16[: ntiles * P]
    return np.tile(out, (8, 1))


def build_blob(meta, core, ebias_l):
    """Per-layer per-block packed int16 blob:
    [idx_c0 .. idx_c{n-1} | mT (bf16 one-hot) | dl (bf16) | ebias (bf16)]."""
    blk, nch = meta["blk"], meta["nch"]
    Tc, T, toff = meta["Tc"], meta["T"], meta["toff"]
    cols = []
    offs = []
    o = 0
    for b in range(blk):
        t = int(T[b])
        parts = []
        for i in range(nch):
            parts.append(wrap16(core["idxc"][i][b], int(Tc[b, i])))
        dlv = core["dl"][:, toff[b]:toff[b] + t]           # [P, t]
        pp, tt = np.nonzero(dlv >= 0)
        mT = np.zeros((P, t * P), BF_NP)
        mT[dlv[pp, tt].astype(np.int64), tt * P + pp] = 1.0
        parts.append(mT.view(np.int16))
        parts.append(dlv.astype(BF_NP).view(np.int16))
        parts.append(ebias_l[:, toff[b] * 8:(toff[b] + t) * 8].view(np.int16))
        blob_b = np.concatenate(parts, axis=1)
        cols.append(blob_b)
        offs.append((o, tuple(int(Tc[b, i]) for i in range(nch)),
                     tuple(int(meta["nmax"][b, i]) for i in range(nch)), t))
        o += blob_b.shape[1]
    return np.concatenate(cols, axis=1), offs


# ------------------------------------------------------------- device build

def build_program(N, meta, blob_w, blob_offs):
    nsh = meta["nsh"]
    blk = meta["blk"]
    bounds = meta["bounds"]
    nch = meta["nch"]
    rows_c = meta["rows_c"]
    segs = meta["segs"]
    nsh_pad = blk * P
    Tmax = int(meta["T"].max())
    rg = [list(range(NCORES))]

    nc = bacc.Bacc("TRN2", target_bir_lowering=False, debug=False,
                   num_devices=NCORES)

    # ---- I/O (weights host-converted to bf16)
    xT = nc.dram_tensor("xT", [F_IN, nsh_pad], BF, kind="ExternalInput")
    blob = [nc.dram_tensor(f"blob{l}", [P, blob_w], I16, kind="ExternalInput")
            for l in range(L)]
    iota_in = nc.dram_tensor("iota_in", [P, P], BF, kind="ExternalInput")
    w_in = nc.dram_tensor("w_in", [F_IN, H], BF, kind="ExternalInput")
    wq = nc.dram_tensor("wq", [L, H, H], BF, kind="ExternalInput")
    wk = nc.dram_tensor("wk", [L, H, H], BF, kind="ExternalInput")
    wv = nc.dram_tensor("wv", [L, H, H], BF, kind="ExternalInput")
    wm = nc.dram_tensor("wm", [L, 2 * H, H], BF, kind="ExternalInput")
    wh1 = nc.dram_tensor("wh1", [H, P], BF, kind="ExternalInput")
    wh2 = nc.dram_tensor("wh2", [P, 1], BF, kind="ExternalInput")
    # rows_in: 0:b_in 1:bo0 2:bm0 3:g0 4:bo1 5:bm1 6:g1 7:bo2 8:bm2 9:g2
    # rows2:   0:beta0 1:beta1 2:beta2 3:bh1(cols 0:P) 4:bh2(col 0)
    rows_in = nc.dram_tensor("rows_in", [1, 10 * H], FP, kind="ExternalInput")
    rows2 = nc.dram_tensor("rows2", [1, 5 * H], FP, kind="ExternalInput")
    y = nc.dram_tensor("y", [nsh_pad, 1], FP, kind="ExternalOutput")

    with tile.TileContext(nc) as tc:
        with (
            tc.tile_pool(name="sbw", bufs=1) as sbw,       # persistent
            tc.tile_pool(name="sbd", bufs=2) as sbd,       # dense working tiles
            tc.tile_pool(name="sbg", bufs=2) as sbg,       # per-block compute
            tc.tile_pool(name="sbb", bufs=4) as sbb,       # blob landing tiles
            tc.tile_pool(name="sbi", bufs=3) as sbi,       # gather landing tiles
            tc.tile_pool(name="dram", bufs=1, space="DRAM") as dram,
            tc.tile_pool(name="p_acc", bufs=2, space="PSUM") as p_acc,
            tc.tile_pool(name="p_big", bufs=2, space="PSUM") as p_big,
            tc.tile_pool(name="p_qg", bufs=2, space="PSUM") as p_qg,
            tc.tile_pool(name="p_tr", bufs=2, space="PSUM") as p_tr,
        ):
            # ---- persistent SBUF constants
            identb = sbw.tile([P, P], BF)
            make_identity(nc, identb[:])
            iota_sb = sbw.tile([P, P], BF)
            nc.sync.dma_start(iota_sb[:], iota_in[:])
            eps_col = sbw.tile([P, 1], FP)
            nc.vector.memset(eps_col[:], EPS_LN)

            w_in_sb = sbw.tile([F_IN, H], BF)
            nc.sync.dma_start(w_in_sb[:], w_in[:])

            def load_chunks(t, l, n_chunks, tag):
                out = []
                for kc in range(n_chunks):
                    s = sbw.tile([P, t.shape[-1]], BF, name=f"{tag}{l}_{kc}",
                                 tag=f"{tag}{l}_{kc}")
                    if l is None:
                        nc.sync.dma_start(s[:], t[kc * P:(kc + 1) * P, :])
                    else:
                        nc.sync.dma_start(s[:], t[l, kc * P:(kc + 1) * P, :])
                    out.append(s)
                return out

            wq_sb = [load_chunks(wq, l, HC, "wq") for l in range(L)]
            wk_sb = [load_chunks(wk, l, HC, "wk") for l in range(L)]
            wv_sb = [load_chunks(wv, l, HC, "wv") for l in range(L)]
            wm_sb = [load_chunks(wm, l, 2 * HC, "wm") for l in range(L)]
            wh1_sb = load_chunks(wh1, None, HC, "wh1")
            wh2_sb = sbw.tile([P, 1], BF)
            nc.sync.dma_start(wh2_sb[:], wh2[:])

            ones_f = sbw.tile([1, P], FP)
            nc.vector.memset(ones_f[:], 1.0)
            rows_sb = sbw.tile([1, 10 * H], FP)
            nc.sync.dma_start(rows_sb[:], rows_in[:])
            rows2_sb = sbw.tile([1, 5 * H], FP)
            nc.sync.dma_start(rows2_sb[:], rows2[:])

            def replicate_row(src_sb, i, tag):
                ps = p_big.tile([P, H], FP, name="pq", tag="pq")
                nc.tensor.matmul(ps[:], lhsT=ones_f[:],
                                 rhs=src_sb[:, i * H:(i + 1) * H],
                                 start=True, stop=True)
                t = sbw.tile([P, H], BF, name=tag, tag=tag)
                nc.scalar.copy(t[:], ps[:])
                return t

            b_in_rep = replicate_row(rows_sb, 0, "b_in_rep")
            bm_rep = [replicate_row(rows_sb, 2 + 3 * l, f"bm_rep{l}")
                      for l in range(L)]
            gb_rep = [replicate_row(rows_sb, 3 + 3 * l, f"g_rep{l}")
                      for l in range(L)]
            bet_rep = [replicate_row(rows2_sb, ll, f"bet_rep{ll}")
                       for ll in range(L)]
            bh_rep = replicate_row(rows2_sb, 3, "bh_rep")
            bh2_rep = replicate_row(rows2_sb, 4, "bh2_rep")

            # resident h^T (bf16) chunks: [128, nsh_pad] each, and Q table
            hT = [sbw.tile([P, nsh_pad], BF, name=f"hT{kc}", tag=f"hT{kc}")
                  for kc in range(HC)]
            qtab_sb = sbw.tile([P, nsh_pad * HC], BF, name="qtab", tag="qtab")

            # ---- internal DRAM
            hdr = dram.tile([nsh_pad, H], FP)
            kvcb = dram.tile([nsh_pad, 2 * H], F8)
            kvfull = [[dram.tile([NCORES * rows_c[i], 2 * H], F8,
                                 addr_space="Shared", name=f"kvf{l}_{i}",
                                 tag=f"kvf{l}_{i}") for i in range(nch)]
                      for l in range(L)]

            # ---------------- helpers
            def transpose_cp(dst_bf_ap, src_bf_ap):
                pt = p_tr.tile([P, P], BF, name="ptr", tag="ptr")
                nc.tensor.transpose(out=pt[:], in_=src_bf_ap, identity=identb[:])
                nc.scalar.copy(dst_bf_ap, pt[:])

            def qkv_block(l, b):
                q_ps = p_big.tile([P, 2 * H], FP, name="pq", tag="pq")
                for kc in range(HC):
                    nc.tensor.matmul(q_ps[:, 0:H],
                                     lhsT=hT[kc][:, b * P:(b + 1) * P],
                                     rhs=wq_sb[l][kc][:],
                                     start=(kc == 0), stop=(kc == HC - 1))
                nc.scalar.copy(qtab_sb[:, b * H:(b + 1) * H], q_ps[:, 0:H])
                # K then V as ONE accumulation group in one bank
                kv_ps = p_big.tile([P, 2 * H], FP, name="pkv", tag="pq")
                for kc in range(HC):
                    nc.tensor.matmul(kv_ps[:, 0:H],
                                     lhsT=hT[kc][:, b * P:(b + 1) * P],
                                     rhs=wk_sb[l][kc][:],
                                     start=(kc == 0), stop=False)
                for kc in range(HC):
                    nc.tensor.matmul(kv_ps[:, H:2 * H],
                                     lhsT=hT[kc][:, b * P:(b + 1) * P],
                                     rhs=wv_sb[l][kc][:],
                                     start=False, stop=(kc == HC - 1))
                kvsb = sbd.tile([P, 2 * H], F8, name="kvsb", tag="kvsb")
                nc.scalar.copy(kvsb[:], kv_ps[:])
                nc.sync.dma_start(kvcb[b * P:(b + 1) * P, :], kvsb[:])

            def allgather_chunks(l, b):
                for i in range(nch):
                    base = 0
                    for (s0, s1) in segs[i]:
                        seg_rows = (s1 - s0) * P
                        if b == s1 - 1:
                            nc.gpsimd.collective_compute(
                                "AllGather", OP.bypass,
                                ins=[kvcb[s0 * P:s1 * P, :].opt()],
                                outs=[kvfull[l][i]
                                      [base:base + NCORES * seg_rows, :].opt()],
                                replica_groups=rg)
                        base += NCORES * seg_rows

            def update_hT(b, h_bf_tile):
                for kc in range(HC):
                    transpose_cp(hT[kc][:, b * P:(b + 1) * P],
                                 h_bf_tile[:, kc * P:(kc + 1) * P])

            # ---------------- phase 0: input projection + QKV(0) + AG(0)
            for b in range(blk):
                xt = sbd.tile([F_IN, P], BF, name="xt", tag="xt")
                nc.sync.dma_start(xt[:], xT[:, b * P:(b + 1) * P])
                ps = p_big.tile([P, H], FP, name="pq", tag="pq")
                nc.tensor.matmul(ps[:], lhsT=xt[:], rhs=w_in_sb[:],
                                 start=True, stop=True)
                t0 = sbd.tile([P, H], FP, name="t0", tag="t0")
                nc.vector.tensor_tensor(t0[:], ps[:], b_in_rep[:], op=OP.add)
                h0 = sbd.tile([P, H], FP, name="h0", tag="sf2")
                nc.vector.tensor_scalar_max(h0[:], t0[:], 0.0)
                nc.sync.dma_start(hdr[b * P:(b + 1) * P, :], h0[:])
                h0b = sbd.tile([P, H], BF, name="h0b", tag="h0b")
                nc.scalar.copy(h0b[:], h0[:])
                update_hT(b, h0b)
                qkv_block(0, b)
                allgather_chunks(0, b)

            # ---------------- layers
            for l in range(L):
                last = (l == L - 1)
                for b in range(blk):
                    o, tc_b, nm_b, t = blob_offs[b]
                    oc = []
                    cur = 0
                    for i in range(nch):
                        oc.append(cur)
                        cur += tc_b[i] * 8
                    oMT = cur
                    oD = oMT + t * P
                    oE = oD + t
                    w_b = oE + t * 8
                    bl = sbb.tile([P, w_b], I16, name="bl", tag="bl")
                    nc.sync.dma_start(bl[:], blob[l][:, o:o + w_b])

                    # one dma_gather per (block, chunk); split only if a
                    # segment exceeds 8 tiles (1024 indices / call)
                    def gather(out3, tlo, nt, nm, in_ap, icol, elem):
                        nchk = -(-nt // 8)
                        done = 0
                        for c in range(nchk):
                            cn = (nt - done + nchk - c - 1) // (nchk - c)
                            c0 = done
                            done += cn
                            nc.gpsimd.dma_gather(
                                out_ap=out3[:, tlo + c0:tlo + c0 + cn, :],
                                in_ap=in_ap,
                                idxs_ap=bl[:, icol + c0 * 8:icol + (c0 + cn) * 8],
                                num_idxs=cn * P, num_idxs_reg=cn * P,
                                elem_size=elem)

                    kvg = sbi.tile([P, t, 2 * H], F8, name="kvg", tag="kvg")
                    tlo = 0
                    for i in range(nch):
                        gather(kvg, tlo, tc_b[i], nm_b[i], kvfull[l][i][:],
                               oc[i], 2 * H)
                        tlo += tc_b[i]

                    # m matrix [P, t*128] bf16 (slot -> dst one-hot rows)
                    m = sbg.tile([P, t * P], BF, name="m", tag="m")
                    nc.vector.tensor_tensor(
                        m[:].rearrange("p (t d) -> p t d", d=P),
                        bl[:, oD:oD + t].bitcast(BF)
                            .rearrange("p (t o) -> p t o", o=1)
                            .to_broadcast([P, t, P]),
                        iota_sb[:].rearrange("p (o d) -> p o d", o=1)
                            .to_broadcast([P, t, P]),
                        op=OP.is_equal)

                    # Q broadcast: qg[slot] = Q_block[dl[slot]] via PE matmul
                    # against the host-built one-hot mT, 2 tiles per PSUM bank
                    qga = sbg.tile([P, t * H], BF, name="qga", tag="qga")
                    for g in range(0, t, 2):
                        gn = min(2, t - g)
                        qp = p_qg.tile([P, gn * H], FP, name="qp", tag="qp")
                        for k in range(gn):
                            nc.tensor.matmul(
                                qp[:, k * H:(k + 1) * H],
                                lhsT=bl[:, oMT + (g + k) * P:
                                        oMT + (g + k + 1) * P].bitcast(BF),
                                rhs=qtab_sb[:, b * H:(b + 1) * H],
                                start=True, stop=True)
                        nc.scalar.copy(qga[:, g * H:(g + gn) * H], qp[:])

                    # qk products -> per-head dots -> exp -> weighted V, in
                    # TWO half-block stages so the ACT exp of half 0 overlaps
                    # the DVE multiplies of half 1 (shorter cross-engine
                    # ping-pong on the critical path)
                    wvq = sbg.tile([P, t * H], BF, name="wvq", tag="wvq")
                    dots = sbg.tile([P, t * NH], BF, name="dots", tag="dots")
                    lg = sbg.tile([P, t * NH], FP, name="lg", tag="lg")
                    lg2 = sbg.tile([P, t * NH], FP, name="lg2", tag="lg2")
                    expt = sbg.tile([P, t, H], BF, name="expt", tag="wvq")
                    wve = sbg.tile([P, t, H + NH], BF, name="wve", tag="qga")
                    acc = p_acc.tile([P, H + NH], FP, name="pacc", tag="pacc")
                    th = (t + 1) // 2
                    for ha, hb in ((0, th), (th, t)):
                        hn = hb - ha
                        if hn <= 0:
                            continue
                        nc.vector.tensor_tensor(
                            wvq[:, ha * H:hb * H]
                                .rearrange("p (t c) -> p t c", c=H),
                            qga[:, ha * H:hb * H]
                                .rearrange("p (t c) -> p t c", c=H),
                            kvg[:, ha:hb, 0:H], op=OP.mult)
                        with nc.allow_low_precision("attn logits fit bf16"):
                            nc.vector.reduce_sum(
                                dots[:, ha * NH:hb * NH]
                                    .rearrange("p (g o) -> p g o", o=1),
                                wvq[:, ha * H:hb * H]
                                    .rearrange("p (g d) -> p g d", d=HD),
                                axis=mybir.AxisListType.X)
                        nc.vector.tensor_tensor(
                            lg[:, ha * NH:hb * NH], dots[:, ha * NH:hb * NH],
                            bl[:, oE + ha * NH:oE + hb * NH].bitcast(BF),
                            op=OP.add)
                        nc.vector.scalar_tensor_tensor(
                            lg2[:, ha * NH:hb * NH],
                            in0=lg[:, ha * NH:hb * NH], scalar=0.2,
                            in1=lg[:, ha * NH:hb * NH], op0=OP.mult,
                            op1=OP.max)
                        # exp expanded across head dim on ACT (bf16 tile)
                        nc.scalar.activation(
                            expt[:, ha:hb, :]
                                .rearrange("p t (h d) -> p t h d", d=HD),
                            lg2[:, ha * NH:hb * NH]
                                .rearrange("p (t h o) -> p t h o", h=NH, o=1)
                            .to_broadcast([P, hn, NH, HD]), AF.Exp)
                        # weighted V and exp(attn) share one tile so the
                        # segment-sum is a single matmul per 128-edge tile
                        nc.scalar.activation(
                            wve[:, ha:hb, H:H + NH],
                            lg2[:, ha * NH:hb * NH]
                                .rearrange("p (t h) -> p t h", h=NH), AF.Exp)
                        nc.vector.tensor_tensor(
                            wve[:, ha:hb, 0:H], kvg[:, ha:hb, H:2 * H],
                            expt[:, ha:hb, :], op=OP.mult)
                        for ti in range(ha, hb):
                            nc.tensor.matmul(acc[:],
                                             lhsT=m[:, ti * P:(ti + 1) * P],
                                             rhs=wve[:, ti, :],
                                             start=(ti == 0),
                                             stop=(ti == t - 1))

                    # drain + normalize
                    ssum = sbd.tile([P, NH], FP, name="ssum", tag="ssum")
                    nc.vector.tensor_scalar_max(ssum[:], acc[:, H:H + NH], 1e-12)
                    rs = sbd.tile([P, NH], FP, name="rs", tag="rs")
                    nc.vector.reciprocal(rs[:], ssum[:])
                    aggb = sbd.tile([P, H], BF, name="aggb", tag="aggb")
                    nc.vector.tensor_tensor(
                        aggb[:].rearrange("p (h d) -> p h d", d=HD),
                        acc[:, 0:H].rearrange("p (h d) -> p h d", d=HD),
                        rs[:].rearrange("p (h o) -> p h o", o=1)
                            .to_broadcast([P, NH, HD]),
                        op=OP.mult)

                    # Wm on [h | agg]: Wo and bo are folded into the second
                    # half of Wm on the host (Wm2' = Wo @ Wm2, bm' = bm +
                    # bo @ Wm2) - linear before the ReLU, and it removes the
                    # Wo matmul, one transpose pair, and an add from the
                    # critical post-phase chain
                    aT = sbd.tile([P, H], BF, name="aT", tag="aT")
                    for kc in range(HC):
                        transpose_cp(aT[:, kc * P:(kc + 1) * P],
                                     aggb[:, kc * P:(kc + 1) * P])
                    wm_ps = p_big.tile([P, H], FP, name="pq", tag="pq")
                    for kc in range(HC):
                        nc.tensor.matmul(wm_ps[:],
                                         lhsT=hT[kc][:, b * P:(b + 1) * P],
                                         rhs=wm_sb[l][kc][:],
                                         start=(kc == 0), stop=False)
                    for kc in range(HC):
                        nc.tensor.matmul(wm_ps[:],
                                         lhsT=aT[:, kc * P:(kc + 1) * P],
                                         rhs=wm_sb[l][HC + kc][:],
                                         start=False, stop=(kc == HC - 1))
                    tm = sbd.tile([P, H], FP, name="tm", tag="sf1")
                    nc.vector.tensor_tensor(tm[:], wm_ps[:], bm_rep[l][:],
                                            op=OP.add)
                    upd = sbd.tile([P, H], FP, name="upd", tag="upd")
                    nc.vector.tensor_scalar_max(upd[:], tm[:], 0.0)

                    # residual + LN (mean/var via bn_stats+bn_aggr in 2 DVE
                    # ops; (x-mu)*rstd fused on ScalarE)
                    h_old = sbd.tile([P, H], FP, name="h_old", tag="h_old")
                    nc.sync.dma_start(h_old[:], hdr[b * P:(b + 1) * P, :])
                    resid = sbd.tile([P, H], FP, name="resid", tag="resid")
                    nc.vector.tensor_tensor(resid[:], h_old[:], upd[:], op=OP.add)
                    bns = sbd.tile([P, 6], FP, name="bns", tag="bns")
                    nc.vector.bn_stats(bns[:], resid[:])
                    mv = sbd.tile([P, 2], FP, name="mv", tag="mv")
                    nc.vector.bn_aggr(mv[:], bns[:])
                    mus = mv[:, 0:1]
                    vpe = mv[:, 1:2]
                    sd = sbd.tile([P, 1], FP, name="sd", tag="sd")
                    nc.scalar.activation(sd[:], vpe, AF.Sqrt,
                                         bias=eps_col[:])
                    rstd = sbd.tile([P, 1], FP, name="rstd", tag="rstd")
                    nc.vector.reciprocal(rstd[:], sd[:])
                    nmr = sbd.tile([P, 1], FP, name="nmr", tag="nmr")
                    nc.vector.scalar_tensor_tensor(nmr[:], in0=mus,
                                                   scalar=-1.0, in1=rstd[:],
                                                   op0=OP.mult, op1=OP.mult)
                    normed = sbd.tile([P, H], FP, name="normed", tag="sf2")
                    nc.scalar.activation(normed[:], resid[:], AF.Identity,
                                         bias=nmr[:], scale=rstd[:])
                    hg = sbd.tile([P, H], FP, name="hg", tag="sf1")
                    nc.vector.tensor_tensor(hg[:], normed[:], gb_rep[l][:],
                                            op=OP.mult)
                    h_new = sbd.tile([P, H], FP, name="h_new", tag="h_new")
                    nc.vector.tensor_tensor(h_new[:], hg[:], bet_rep[l][:],
                                            op=OP.add)

                    hb = sbd.tile([P, H], BF, name="hb", tag="hb")
                    nc.scalar.copy(hb[:], h_new[:])
                    update_hT(b, hb)
                    if not last:
                        nc.sync.dma_start(hdr[b * P:(b + 1) * P, :], h_new[:])
                        qkv_block(l + 1, b)
                        allgather_chunks(l + 1, b)
                    else:
                        # output head
                        h1_ps = p_big.tile([P, P], FP, name="pq", tag="pq")
                        for kc in range(HC):
                            nc.tensor.matmul(h1_ps[:],
                                             lhsT=hT[kc][:, b * P:(b + 1) * P],
                                             rhs=wh1_sb[kc][:],
                                             start=(kc == 0), stop=(kc == HC - 1))
                        t1s = sbd.tile([P, P], FP, name="t1s", tag="t1s")
                        nc.vector.tensor_tensor(t1s[:], h1_ps[:],
                                                bh_rep[:, 0:P], op=OP.add)
                        t1 = sbd.tile([P, P], BF, name="t1", tag="t1")
                        nc.vector.tensor_scalar_max(t1[:], t1s[:], 0.0)
                        t1T = sbd.tile([P, P], BF, name="t1T", tag="t1T")
                        transpose_cp(t1T[:], t1[:])
                        y_ps = p_big.tile([P, 1], FP, name="pq", tag="pq")
                        nc.tensor.matmul(y_ps[:], lhsT=t1T[:], rhs=wh2_sb[:],
                                         start=True, stop=True)
                        yt = sbd.tile([P, 1], FP, name="yt", tag="yt")
                        nc.vector.tensor_tensor(yt[:], y_ps[:], bh2_rep[:, 0:1],
                                                op=OP.add)
                        nc.sync.dma_start(y[b * P:(b + 1) * P, :], yt[:])

    nc.compile()
    return nc


# ------------------------------------------------------------------ driver

def make_in_maps(inputs, meta, cores):
    N = inputs["x"].shape[0]
    nsh = meta["nsh"]
    blk = meta["blk"]
    nsh_pad = blk * P
    Tmax = int(meta["T"].max())
    T_tot = meta["T_tot"]
    x = np.asarray(inputs["x"], np.float32)
    edge_attr = np.asarray(inputs["edge_attr"], np.float32)
    We = np.asarray(inputs["We"], np.float32)
    scale = HD ** -0.5

    def bf(a):
        return np.ascontiguousarray(np.asarray(a, np.float32).astype(BF_NP))

    Wm = np.asarray(inputs["Wm"], np.float32)
    Wo = np.asarray(inputs["Wo"], np.float32)
    bo = np.asarray(inputs["bo"], np.float32)
    bm = np.asarray(inputs["bm"], np.float32)
    wm_f = np.concatenate([Wm[:, :H], Wo @ Wm[:, H:]], axis=1)
    bm_f = bm + np.einsum("lh,lhj->lj", bo, Wm[:, H:])

    rows_in = np.zeros((10, H), np.float32)
    rows_in[0, :] = np.asarray(inputs["b_in"], np.float32)
    for l in range(L):
        rows_in[1 + 3 * l] = np.asarray(inputs["bo"], np.float32)[l]
        rows_in[2 + 3 * l] = bm_f[l]
        rows_in[3 + 3 * l] = np.asarray(inputs["gamma"], np.float32)[l]
    rows_in = rows_in.reshape(1, 10 * H)
    rows2 = np.zeros((5, H), np.float32)
    for l in range(L):
        rows2[l] = np.asarray(inputs["beta"], np.float32)[l]
    rows2[3, 0:P] = np.asarray(inputs["b_h1"], np.float32)
    rows2[4, 0] = float(np.asarray(inputs["b_h2"], np.float32).reshape(-1)[0])
    rows2 = rows2.reshape(1, 5 * H)

    iota = np.tile(np.arange(P, dtype=np.float32)[None, :], (P, 1))

    common = {
        "iota_in": bf(iota),
        "w_in": bf(inputs["W_in"]),
        "wq": bf(np.asarray(inputs["Wq"], np.float32) * scale),
        "wk": bf(inputs["Wk"]),
        "wv": bf(inputs["Wv"]),
        "wm": bf(wm_f),
        "wh1": bf(inputs["W_h1"]),
        "wh2": bf(np.asarray(inputs["W_h2"], np.float32).reshape(P, 1)),
        "rows_in": rows_in,
        "rows2": rows2,
    }

    in_maps = []
    blob_offs = None
    for c in range(NCORES):
        core = cores[c]
        xT = np.zeros((F_IN, nsh_pad), np.float32)
        xT[:, :nsh] = x[c * nsh:(c + 1) * nsh].T
        esel = core["esel"]
        valid = esel >= 0
        m = dict(common)
        m["xT"] = bf(xT)
        for l in range(L):
            eb = np.zeros((T_tot * P, NH), np.float32)
            eb[valid] = edge_attr[esel[valid]] @ We[l]
            ebias = np.ascontiguousarray(
                eb.reshape(T_tot, P, NH).transpose(1, 0, 2).reshape(P, T_tot * NH)
            ).astype(BF_NP)
            blob_arr, offs = build_blob(meta, core, ebias)
            m[f"blob{l}"] = blob_arr
            blob_offs = offs
        in_maps.append(m)
    return in_maps, blob_offs


_BUILD_CACHE = {}
LAST_EXEC_NS = None


def kernel(**inputs) -> np.ndarray:
    global LAST_EXEC_NS
    import os
    edge_index = np.asarray(inputs["edge_index"])
    N = inputs["x"].shape[0]
    nsh = N // NCORES
    meta, cores = edge_schedule(edge_index, N)
    in_maps, blob_offs = make_in_maps(inputs, meta, cores)
    blob_w = in_maps[0]["blob0"].shape[1]
    key = (N, blob_w, tuple(meta["T"].tolist()),
           tuple(map(tuple, meta["Tc"].tolist())))
    if key not in _BUILD_CACHE:
        _BUILD_CACHE[key] = build_program(N, meta, blob_w, blob_offs)
    nc = _BUILD_CACHE[key]
    trace = os.environ.get("KERNEL_TRACE", "0") == "1"
    res = run_bass_kernel_spmd(nc, in_maps, core_ids=list(range(NCORES)),
                               trace=trace)
    if res.exec_time_ns is not None:
        LAST_EXEC_NS = res.exec_time_ns
        tp = res.instructions_and_trace[1] if res.instructions_and_trace else None
        print(f"[kernel] exec_time_ns={res.exec_time_ns} trace={tp}")
    out = np.concatenate([np.asarray(res.results[c]["y"])[:nsh]
                          for c in range(NCORES)], 0)
    return out.astype(np.float32)


# ---------------------------------------------------------------- reference

def np_forward(inp):
    """Numpy port of the jax reference (for --sim self-check)."""
    N = inp["x"].shape[0]
    src = inp["edge_index"][0].astype(np.int64)
    dst = inp["edge_index"][1].astype(np.int64)
    scale = HD ** -0.5
    h = np.maximum(inp["x"] @ inp["W_in"] + inp["b_in"], 0.0)
    for l in range(L):
        Q = (h @ inp["Wq"][l]).reshape(N, NH, HD)
        K = (h @ inp["Wk"][l]).reshape(N, NH, HD)
        V = (h @ inp["Wv"][l]).reshape(N, NH, HD)
        eb = inp["edge_attr"] @ inp["We"][l]
        attn = (Q[dst] * K[src]).sum(-1) * scale + eb
        attn = np.where(attn > 0, attn, 0.2 * attn)
        aexp = np.exp(attn - attn.max())
        asum = np.zeros((N, NH))
        np.add.at(asum, dst, aexp)
        anorm = aexp / np.clip(asum[dst], 1e-12, None)
        wV = V[src] * anorm[..., None]
        agg = np.zeros((N, NH, HD))
        np.add.at(agg, dst, wV)
        agg = agg.reshape(N, H) @ inp["Wo"][l] + inp["bo"][l]
        upd = np.maximum(
            np.concatenate([h, agg], 1) @ inp["Wm"][l] + inp["bm"][l], 0.0)
        hh = h + upd
        mu = hh.mean(-1, keepdims=True)
        var = hh.var(-1, keepdims=True)
        h = (hh - mu) / np.sqrt(var + EPS_LN) * inp["gamma"][l] + inp["beta"][l]
    return np.maximum(h @ inp["W_h1"] + inp["b_h1"], 0.0) @ inp["W_h2"] + inp["b_h2"]


EPS = EPS_LN

if __name__ == "__main__":
    import argparse
    parser = argparse.ArgumentParser()
    parser.add_argument("--sim", action="store_true")
    args = parser.parse_args()
    if args.sim:
        from concourse.bass_interp import MultiCoreSim
        rng = np.random.default_rng(0)
        Nl, El = 4096, 32768
        g = lambda *s: (rng.standard_normal(s) * 0.05).astype(np.float32)
        inp = {
            "x": rng.standard_normal((Nl, F_IN)).astype(np.float32),
            "edge_attr": rng.standard_normal((El, ED)).astype(np.float32),
            "W_in": g(F_IN, H), "b_in": (rng.standard_normal(H) * 0.01).astype(np.float32),
            "Wq": g(L, H, H), "Wk": g(L, H, H), "Wv": g(L, H, H),
            "We": g(L, ED, NH), "Wo": g(L, H, H),
            "bo": (rng.standard_normal((L, H)) * 0.01).astype(np.float32),
            "Wm": g(L, 2 * H, H),
            "bm": (rng.standard_normal((L, H)) * 0.01).astype(np.float32),
            "gamma": (1 + 0.1 * rng.standard_normal((L, H))).astype(np.float32),
            "beta": (0.1 * rng.standard_normal((L, H))).astype(np.float32),
            "W_h1": g(H, H // 2), "b_h1": (rng.standard_normal(H // 2) * 0.01).astype(np.float32),
            "W_h2": g(H // 2, 1), "b_h2": np.zeros(1, np.float32),
            "edge_index": rng.integers(0, Nl, size=(2, El)).astype(np.int64),
        }
        meta, cores = edge_schedule(inp["edge_index"], Nl)
        in_maps, blob_offs = make_in_maps(inp, meta, cores)
        blob_w = in_maps[0]["blob0"].shape[1]
        print(f"sim build: T={meta['T'].tolist()} Tc={meta['Tc'].tolist()} blob_w={blob_w}")
        nc = build_program(Nl, meta, blob_w, blob_offs)
        sim = MultiCoreSim(nc, num_cores=NCORES, num_workers=0)
        for c in range(NCORES):
            for k, v in in_maps[c].items():
                sim.cores[c].tensor(k)[:] = v
        sim.simulate(check_with_hw=False)
        nshl = Nl // NCORES
        got = np.concatenate([np.asarray(sim.cores[c].tensor("y"))[:nshl]
                              for c in range(NCORES)], 0)
        want = np_forward(inp)
        err = np.abs(got - want)
        print(f"sim maxabs={err.max():.3e} rel={err.max()/np.abs(want).max():.3e}")
